# revision 50
# baseline (speedup 1.0000x reference)
"""Trainium2 Bass kernel for GAT+GCN+MLP message passing (8 NeuronCores, SPMD).

Strategy (dst-node sharding), v2:
  - Host: add self-loops, greedily pack the 10000 nodes into 8 cores x 10
    tiles x 128 slots balancing in-edge counts; build per-edge one-hot dst
    masks (plus transposed variant) and the edge-expanded x operands.
  - GAT: per tile, all e-coefficients accumulate into one PSUM strip and go
    through a single Prelu+Exp on the scalar engine; the 670-wide per-edge
    message product (a constant-1 67th feature makes the softmax denominators
    fall out of the same aggregation matmul) is split across the vector and
    pool engines; the normalized aggregate is transposed per head for the
    per-head W matmul and leaky+bias is fused into scalar-engine Prelu ops.
  - The bf16 hidden h (660 wide) is exchanged with piecewise AllGathers
    (3/3/2/2 tiles each, issued as phase B finishes each piece) into row
    slices of one Local DRAM tensor so most of the collective hides under
    GAT compute; GCN gathers h rows with dma_gather (one instruction per
    half-tile, 16-way-wrapped int16 indices replicated per Q7 core, rows
    padded to 768 for the 256B stride), aggregates feature-major
    (gathered rows stationary, on-device norm-scaled masks moving) so no
    transposes are needed, and the dense MLP runs feature-major with
    bias+leaky fused into Prelu activations.
"""

import os
import sys
import heapq
import dataclasses

for _p in ("/opt/trn_rl_repo", "/root/.axon_site/_ro/trn_rl_repo"):
    if os.path.isdir(_p) and _p not in sys.path:
        sys.path.insert(0, _p)

import numpy as np
import ml_dtypes

import concourse.bass as bass
import concourse.tile as tile
from concourse import bacc, mybir
from concourse.bass import IndirectOffsetOnAxis
from concourse.bass_utils import run_bass_kernel_spmd

BF16 = ml_dtypes.bfloat16

N = 10000
F_IN = 66
HEADS = 10
F_HEAD = 66
F_GAT = HEADS * F_HEAD          # 660
GCN_OUT = 1320
NCORE = 8
TILES_PER_CORE = 10
NTILE = NCORE * TILES_PER_CORE  # 80
NSLOT = NTILE * 128             # 10240
SLOTS_PER_CORE = TILES_PER_CORE * 128  # 1280
F67 = F_IN + 1                  # x features + a ones column (denominator trick)
HW = 768                        # h row width: 660 used + pad to a 256B stride
PIECES = (3, 3, 2, 2)           # tiles per piecewise-AllGather piece

F32 = mybir.dt.float32
BF = mybir.dt.bfloat16
I32 = mybir.dt.int32
I16 = mybir.dt.int16

_CACHE = {}


# ---------------------------------------------------------------- host prep

def _prep(x, edge_index):
    src = np.concatenate([edge_index[0], np.arange(N, dtype=np.int64)])
    dst = np.concatenate([edge_index[1], np.arange(N, dtype=np.int64)])
    deg = np.bincount(dst, minlength=N).astype(np.int64)

    # pack nodes into 80 tiles of <=128 slots, balancing in-edge counts
    order = np.argsort(-deg, kind="stable")
    tile_cnt = np.zeros(NTILE, np.int64)
    slot = np.empty(N, np.int64)
    hp = [(0, t) for t in range(NTILE)]
    heapq.heapify(hp)
    for n_ in order:
        while True:
            e, t = heapq.heappop(hp)
            if tile_cnt[t] < 128:
                break
        slot[n_] = t * 128 + tile_cnt[t]
        tile_cnt[t] += 1
        heapq.heappush(hp, (e + int(deg[n_]), t))
    sslot = slot[src]
    dslot = slot[dst]
    dtile = dslot >> 7
    dlocal = dslot & 127

    tile_edges = np.bincount(dtile, minlength=NTILE)
    nc_t = int(np.max((tile_edges + 127) // 128))
    e_tile = nc_t * 128
    nchunks = TILES_PER_CORE * nc_t

    esrc = np.zeros((NCORE, TILES_PER_CORE, e_tile), np.int64)
    edstl = np.full((NCORE, TILES_PER_CORE, e_tile), -1, np.int64)
    edst = np.zeros((NCORE, TILES_PER_CORE, e_tile), np.int64)
    ord_t = np.argsort(dtile, kind="stable")
    bounds = np.searchsorted(dtile[ord_t], np.arange(NTILE + 1))
    for t in range(NTILE):
        idx = ord_t[bounds[t]:bounds[t + 1]]
        k = len(idx)
        c, tt = divmod(t, TILES_PER_CORE)
        esrc[c, tt, :k] = sslot[idx]
        edstl[c, tt, :k] = dlocal[idx]
        edst[c, tt, :k] = dslot[idx]

    # one-hot masks [core][128 edge-part, nchunks*128]
    onehot = (edstl[..., None] == np.arange(128))      # [C,T,e_tile,128] bool
    oh = onehot.reshape(NCORE, TILES_PER_CORE, nc_t, 128, 128)
    masks = np.ascontiguousarray(
        oh.transpose(0, 3, 1, 2, 4)).reshape(
        NCORE, 128, nchunks * 128).astype(BF16)
    masksT = np.ascontiguousarray(
        oh.transpose(0, 4, 1, 2, 3)).reshape(
        NCORE, 128, nchunks * 128).astype(BF16)

    # per-edge GCN norm weights, chunk-major [128, nchunks]
    dinv_slot = np.ones(NSLOT, np.float32)
    dinv_slot[slot] = 1.0 / np.sqrt(np.maximum(deg, 1).astype(np.float32))
    wv = (dinv_slot[esrc] * dinv_slot[edst]).astype(np.float32)
    wvals = np.ascontiguousarray(
        wv.reshape(NCORE, nchunks, 128).transpose(0, 2, 1))

    # edge-expanded x operands; xg carries a constant-1 67th feature so the
    # softmax denominators fall out of the same aggregation matmul
    x_pad = np.zeros((NSLOT, F_IN), np.float32)
    x_pad[slot] = x
    xg = np.empty((NCORE, 128, nchunks * F67), BF16)
    xgT = np.empty((NCORE, F_IN, nchunks * 128), BF16)
    for c in range(NCORE):
        arr = x_pad[esrc[c].reshape(-1)]               # [nidx, 66] f32
        a3 = arr.reshape(nchunks, 128, F_IN)
        a4 = np.concatenate(
            [a3, np.ones((nchunks, 128, 1), np.float32)], axis=2)
        xg[c] = np.ascontiguousarray(
            a4.transpose(1, 0, 2)).reshape(128, nchunks * F67).astype(BF16)
        xgT[c] = np.ascontiguousarray(arr.T).astype(BF16)

    # remap slot ids to the piece-major h_full layout produced by the
    # piecewise AllGather: pieces of PIECES tiles; piece p holds
    # [core0 rows | core1 rows | ...], rows = (tile - first_tile_p)*128+local
    pbase = np.zeros(len(PIECES) + 1, np.int64)
    for p, ntl in enumerate(PIECES):
        pbase[p + 1] = pbase[p] + NCORE * ntl * 128
    tile2piece = np.repeat(np.arange(len(PIECES)), PIECES)
    tile2first = np.concatenate(
        [[sum(PIECES[:p])] * PIECES[p] for p in range(len(PIECES))])

    piece_arr = np.asarray(PIECES, np.int64)

    def remap(slots):
        core = slots // SLOTS_PER_CORE
        tl = (slots % SLOTS_PER_CORE) // 128
        local = slots % 128
        p = tile2piece[tl]
        return (pbase[p] + core * (piece_arr[p] * 128)
                + (tl - tile2first[tl]) * 128 + local)

    # per half-tile idx vectors for dma_gather, chunk-major order, 16-way
    # partition wrap replicated for each of the 8 gpsimd Q7 cores
    h1 = (nc_t + 1) // 2
    cols = [h1 * 128 // 16, (nc_t - h1) * 128 // 16]
    sidx = np.zeros((NCORE, 128, TILES_PER_CORE * sum(cols)), np.int16)
    for c in range(NCORE):
        rm = remap(esrc[c]).reshape(TILES_PER_CORE, nc_t * 128)
        for t in range(TILES_PER_CORE):
            o = t * sum(cols)
            b1 = rm[t, :h1 * 128].reshape(-1, 16).T
            b2 = rm[t, h1 * 128:].reshape(-1, 16).T
            for k in range(8):
                sidx[c, 16 * k:16 * (k + 1), o:o + cols[0]] = b1
                sidx[c, 16 * k:16 * (k + 1),
                     o + cols[0]:o + sum(cols)] = b2

    xT = np.zeros((F_IN, NSLOT), np.float32)
    xT[:, slot] = x.T
    xT_my = np.stack([np.ascontiguousarray(
        xT[:, c * SLOTS_PER_CORE:(c + 1) * SLOTS_PER_CORE])
        for c in range(NCORE)])

    return dict(slot=slot, nc_t=nc_t, masks=masks, masksT=masksT,
                wvals=wvals, xg=xg, xgT=xgT, sidx=sidx, xT_my=xT_my)


def _prep_weights(W_gat, att_src, att_dst, b_gat, W_gcn, b_gcn,
                  W_g1, b_g1, W_g2, b_g2, W_fc1, b_fc1, W_fc2, b_fc2,
                  W_out, b_out):
    Wg = np.asarray(W_gat, np.float32).reshape(F_IN, HEADS, F_HEAD)
    w_as = np.einsum("fhg,hg->fh", Wg, np.asarray(att_src, np.float32))
    w_ad = np.einsum("fhg,hg->fh", Wg, np.asarray(att_dst, np.float32))
    w_asd = np.ascontiguousarray(
        np.concatenate([w_as, w_ad], axis=1).astype(np.float32))  # [66,20]
    w_as_bf = np.ascontiguousarray(w_as.astype(BF16))             # [66,10]

    def chunk_pack(W, kchunks, ncols):
        W = np.asarray(W, np.float32)
        K, M = W.shape
        out = np.zeros((128, kchunks * ncols), BF16)
        for kt in range(kchunks):
            r0 = kt * 128
            r1 = min(K, r0 + 128)
            if r0 >= K:
                break
            out[:r1 - r0, kt * ncols:kt * ncols + M] = W[r0:r1].astype(BF16)
        return out

    W_gcn_p = chunk_pack(W_gcn, 6, GCN_OUT)
    W_g1_p = chunk_pack(W_g1, 11, 1000)
    W_g2_p = chunk_pack(W_g2, 8, 64)

    def col_pack(b, nch):
        b = np.asarray(b, np.float32).reshape(-1)
        out = np.zeros((128, nch), np.float32)
        for mc in range(nch):
            r0 = mc * 128
            r1 = min(b.shape[0], r0 + 128)
            if r0 >= b.shape[0]:
                break
            out[:r1 - r0, mc] = b[r0:r1]
        return out

    b_gcn_col = col_pack(b_gcn, 11)
    b_g1_col = col_pack(b_g1, 8)
    W_fc1_p = np.asarray(W_fc1, BF16)
    W_fc2_p = np.asarray(W_fc2, BF16)
    W_out_p = np.asarray(W_out, BF16)
    b_tail = np.zeros((128, 4), np.float32)
    b_tail[:64, 0] = np.asarray(b_g2, np.float32)
    b_tail[:32, 1] = np.asarray(b_fc1, np.float32)
    b_tail[:16, 2] = np.asarray(b_fc2, np.float32)
    b_tail[0, 3] = float(np.asarray(b_out).reshape(-1)[0])

    ident = np.eye(128, dtype=BF16)
    ones_row = np.ones((1, 128), BF16)
    b_gat_row = np.zeros((1, F_GAT), BF16)
    b_gat_row[0, :] = np.asarray(b_gat, BF16)
    W_heads = np.asarray(W_gat, BF16)

    return dict(w_asd=w_asd, w_as_bf=w_as_bf, W_gcn_p=W_gcn_p, W_g1_p=W_g1_p,
                W_g2_p=W_g2_p, b_gcn_col=b_gcn_col, b_g1_col=b_g1_col,
                W_fc1_p=W_fc1_p, W_fc2_p=W_fc2_p, W_out_p=W_out_p,
                b_tail=b_tail, ident=ident, ones_row=ones_row,
                b_gat_row=b_gat_row, W_heads=W_heads)


# ---------------------------------------------------------------- device kernel

def _bc(ap, pattern):
    """Replace the free dims of a (sliced) AP with explicit [step,count] dims."""
    return dataclasses.replace(
        ap, ap=[list(ap.ap[0])] + [list(p) for p in pattern])


def _build(nc_t, repeat=1, collective=True, use_prelu=True):
    nchunks = TILES_PER_CORE * nc_t

    nc = bacc.Bacc("TRN2", target_bir_lowering=False, debug=False,
                   num_devices=NCORE, dynamic_dma_scratch_size=1 << 16)

    def inp(name, shape, dt):
        return nc.dram_tensor(name, list(shape), dt, kind="ExternalInput")

    xg_d = inp("xg", [128, nchunks * F67], BF)
    xgT_d = inp("xgT", [F_IN, nchunks * 128], BF)
    masks_d = inp("masks", [128, nchunks * 128], BF)
    masksT_d = inp("masksT", [128, nchunks * 128], BF)
    wvals_d = inp("wvals", [128, nchunks], F32)
    h1 = (nc_t + 1) // 2
    gcols = [h1 * 128 // 16, (nc_t - h1) * 128 // 16]
    sidx_d = inp("sidx", [128, TILES_PER_CORE * sum(gcols)], I16)
    xTmy_d = inp("xT_my", [F_IN, SLOTS_PER_CORE], F32)
    w_asd_d = inp("w_asd", [F_IN, 2 * HEADS], F32)
    w_as_bf_d = inp("w_as_bf", [F_IN, HEADS], BF)
    W_heads_d = inp("W_heads", [F_IN, F_GAT], BF)
    b_gat_row_d = inp("b_gat_row", [1, F_GAT], BF)
    ones_row_d = inp("ones_row", [1, 128], BF)
    ident_d = inp("ident", [128, 128], BF)
    W_gcn_d = inp("W_gcn_p", [128, 6 * GCN_OUT], BF)
    W_g1_d = inp("W_g1_p", [128, 11 * 1000], BF)
    W_g2_d = inp("W_g2_p", [128, 8 * 64], BF)
    b_gcn_col_d = inp("b_gcn_col", [128, 11], F32)
    b_g1_col_d = inp("b_g1_col", [128, 8], F32)
    W_fc1_d = inp("W_fc1_p", [64, 32], BF)
    W_fc2_d = inp("W_fc2_p", [32, 16], BF)
    W_out_d = inp("W_out_p", [16, 1], BF)
    b_tail_d = inp("b_tail", [128, 4], F32)

    y_d = nc.dram_tensor("y", [1, SLOTS_PER_CORE], F32, kind="ExternalOutput")

    core_ids = list(range(NCORE))
    AF = mybir.ActivationFunctionType
    OP = mybir.AluOpType

    with tile.TileContext(nc) as tc:
        with tc.tile_pool(name="persist", bufs=1) as pp, \
             tc.tile_pool(name="dram", bufs=1, space="DRAM") as dram:

            al001 = pp.tile([128, 1], F32)
            nc.vector.memset(al001[:], 0.01)
            al02 = pp.tile([128, 1], F32)
            nc.vector.memset(al02[:], 0.2)

            def leaky_act(out_ap, in_ap, alpha_ap, pool, bias=None, nparts=128):
                """out = leaky(in + bias); bias is a per-partition AP or None."""
                if use_prelu:
                    kw = {}
                    if bias is not None:
                        kw["bias"] = bias
                    nc.scalar.activation(out_ap, in_ap, AF.Prelu,
                                         alpha=alpha_ap[0:nparts, 0:1], **kw)
                else:
                    fs = in_ap.free_size()
                    th = pool.tile([128, fs], BF, tag="lk_t", name="lk_t")
                    if bias is not None:
                        nc.scalar.activation(th[0:nparts, 0:fs], in_ap,
                                             AF.Identity, bias=bias)
                    else:
                        nc.scalar.activation(th[0:nparts, 0:fs], in_ap, AF.Copy)
                    u = pool.tile([128, fs], BF, tag="lk_u", name="lk_u")
                    alpha = 0.2 if alpha_ap is al02 else 0.01
                    nc.vector.tensor_scalar(u[0:nparts, 0:fs], th[0:nparts, 0:fs],
                                            alpha, None, OP.mult)
                    nc.vector.tensor_tensor(out_ap, th[0:nparts, 0:fs],
                                            u[0:nparts, 0:fs], OP.max)

            sidx_sb = pp.tile([128, TILES_PER_CORE * sum(gcols)], I16)
            ident_sb = pp.tile([128, 128], BF)
            ones_sb = pp.tile([1, 128], BF)
            wvals_sb = pp.tile([128, nchunks], F32)
            asd_my_bf = pp.tile([128, TILES_PER_CORE * 2 * HEADS], BF)
            nc.sync.dma_start(sidx_sb[:], sidx_d[:])
            nc.sync.dma_start(ident_sb[:], ident_d[:])
            nc.sync.dma_start(ones_sb[:], ones_row_d[:])
            nc.sync.dma_start(wvals_sb[:], wvals_d[:])

            # phase-C weights: load early on the vector queue so they stream
            # behind the phase-B inputs
            W_gcn_sb = pp.tile([128, 6 * GCN_OUT], BF)
            nc.scalar.dma_start(W_gcn_sb[:], W_gcn_d[:])
            W_g2_sb = pp.tile([128, 8 * 64], BF)
            nc.scalar.dma_start(W_g2_sb[:], W_g2_d[:])
            b_gcn_sb = pp.tile([128, 11], F32)
            nc.scalar.dma_start(b_gcn_sb[:], b_gcn_col_d[:])
            b_g1_sb = pp.tile([128, 8], F32)
            nc.scalar.dma_start(b_g1_sb[:], b_g1_col_d[:])
            W_fc1_sb = pp.tile([64, 32], BF)
            nc.scalar.dma_start(W_fc1_sb[:], W_fc1_d[:])
            W_fc2_sb = pp.tile([32, 16], BF)
            nc.scalar.dma_start(W_fc2_sb[:], W_fc2_d[:])
            W_out_sb = pp.tile([16, 1], BF)
            nc.scalar.dma_start(W_out_sb[:], W_out_d[:])
            b_tail_sb = pp.tile([128, 4], F32)
            nc.scalar.dma_start(b_tail_sb[:], b_tail_d[:])

            # masks persist through both phases (phase C derives wmasks);
            # loaded tile-interleaved alongside the phase-B inputs below
            masks_sb = pp.tile([128, nchunks * 128], BF)

            for _rep in range(repeat):
                h_my = dram.tile([SLOTS_PER_CORE, HW], BF,
                                 name=f"h_my_{_rep}")
                h_full = dram.tile([NSLOT, HW], BF,
                                   name=f"h_full_{_rep}")
                pb = 0
                h_fp = []
                for p, ntl in enumerate(PIECES):
                    h_fp.append(h_full[pb:pb + NCORE * ntl * 128, :])
                    pb += NCORE * ntl * 128
                # ---------------- phase A: own-node attention coefficients ----
                with tc.tile_pool(name="phaseA", bufs=1) as pa, \
                     tc.tile_pool(name="psumA", bufs=4,
                                  space=bass.MemorySpace.PSUM) as psa:
                    xTmy_sb = pa.tile([F_IN, SLOTS_PER_CORE], F32)
                    nc.sync.dma_start(xTmy_sb[:], xTmy_d[:])
                    wasd_sb = pa.tile([F_IN, 2 * HEADS], F32)
                    nc.sync.dma_start(wasd_sb[:], w_asd_d[:])
                    for t in range(TILES_PER_CORE):
                        ps = psa.tile([128, 2 * HEADS], F32, tag="psA")
                        nc.tensor.matmul(ps[:], xTmy_sb[:, 128 * t:128 * (t + 1)],
                                         wasd_sb[:], start=True, stop=True)
                        nc.vector.tensor_copy(
                            asd_my_bf[:, 20 * t:20 * (t + 1)], ps[:])

                # ---------------- phase B: GAT ----------------
                with tc.tile_pool(name="phaseB", bufs=1) as pb, \
                     tc.tile_pool(name="gat_work",
                                  bufs=4 if use_prelu else 3) as gw, \
                     tc.tile_pool(name="lk_b", bufs=2) as lkb, \
                     tc.tile_pool(name="psumE", bufs=1,
                                  space=bass.MemorySpace.PSUM) as pse, \
                     tc.tile_pool(name="psumG", bufs=2,
                                  space=bass.MemorySpace.PSUM) as psg, \
                     tc.tile_pool(name="psumH", bufs=1,
                                  space=bass.MemorySpace.PSUM) as psh, \
                     tc.tile_pool(name="psumT", bufs=1,
                                  space=bass.MemorySpace.PSUM) as pst:

                    xg_sb = pb.tile([128, nchunks * F67], BF)
                    xgT_sb = pb.tile([F_IN, nchunks * 128], BF)
                    masksT_sb = pb.tile([128, nchunks * 128], BF)
                    was_sb = pb.tile([F_IN, HEADS], BF)
                    nc.sync.dma_start(was_sb[:], w_as_bf_d[:])
                    W_heads_sb = pb.tile([F_IN, F_GAT], BF)
                    nc.sync.dma_start(W_heads_sb[:], W_heads_d[:])
                    b_gat_sb = pb.tile([1, F_GAT], BF)
                    nc.sync.dma_start(b_gat_sb[:], b_gat_row_d[:])
                    for t in range(TILES_PER_CORE):
                        sl = slice(128 * nc_t * t, 128 * nc_t * (t + 1))
                        nc.sync.dma_start(xgT_sb[:, sl], xgT_d[:, sl])
                        nc.sync.dma_start(masksT_sb[:, sl], masksT_d[:, sl])
                        slx = slice(F67 * nc_t * t, F67 * nc_t * (t + 1))
                        nc.sync.dma_start(xg_sb[:, slx], xg_d[:, slx])
                        nc.sync.dma_start(
                            masks_sb[:, sl],
                            masks_d[:, sl])
                    exb = pb.tile([128, nchunks * HEADS], BF)

                    for t in range(TILES_PER_CORE):
                        a_d_tile = asd_my_bf[:, 20 * t + 10:20 * (t + 1)]
                        # e-values for the whole tile in one PSUM strip
                        ps_e = pse.tile([128, nc_t * HEADS], F32, tag="pse")
                        for k in range(nc_t):
                            c = t * nc_t + k
                            nc.tensor.matmul(ps_e[:, 10 * k:10 * (k + 1)],
                                             xgT_sb[:, 128 * c:128 * (c + 1)],
                                             was_sb[:], start=True, stop=False)
                            nc.tensor.matmul(ps_e[:, 10 * k:10 * (k + 1)],
                                             masksT_sb[:, 128 * c:128 * (c + 1)],
                                             a_d_tile, start=False, stop=True)
                        ev = gw.tile([128, nc_t * HEADS], F32, tag="ev")
                        if use_prelu:
                            nc.scalar.activation(ev[:], ps_e[:], AF.Prelu,
                                                 alpha=al02[:, 0:1])
                        else:
                            nc.vector.tensor_scalar(ev[:], ps_e[:], 0.2, None,
                                                    OP.mult)
                            nc.vector.tensor_tensor(ev[:], ps_e[:], ev[:], OP.max)
                        nc.scalar.activation(
                            exb[:, 10 * nc_t * t:10 * nc_t * (t + 1)], ev[:],
                            AF.Exp)

                        # psum_g: aggregate at cols [0:660), denominators [660:670)
                        psum_g = psg.tile([128, 670], F32, tag="psg")
                        for k in range(nc_t):
                            c = t * nc_t + k
                            rhs = gw.tile([128, 10 * F67], BF, tag="rhs")
                            xg_b1 = _bc(xg_sb[:, F67 * c:F67 * c + 1],
                                        [[0, 7], [1, F67]])
                            ex_b1 = _bc(exb[:, HEADS * c:HEADS * c + 1],
                                        [[1, 7], [0, F67]])
                            nc.vector.tensor_tensor(
                                rhs[:, 0:469].rearrange("p (h f) -> p h f", h=7),
                                xg_b1, ex_b1, OP.mult)
                            xg_b2 = _bc(xg_sb[:, F67 * c:F67 * c + 1],
                                        [[0, 3], [1, F67]])
                            ex_b2 = _bc(exb[:, HEADS * c + 7:HEADS * c + 8],
                                        [[1, 3], [0, F67]])
                            nc.gpsimd.tensor_tensor(
                                rhs[:, 469:670].rearrange("p (h f) -> p h f", h=3),
                                xg_b2, ex_b2, OP.mult)
                            mask = masks_sb[:, 128 * c:128 * (c + 1)]
                            st, sp = (k == 0), (k == nc_t - 1)
                            nc.tensor.matmul(psum_g[:, 0:512], mask,
                                             rhs[:, 0:512], start=st, stop=sp)
                            nc.tensor.matmul(psum_g[:, 512:670], mask,
                                             rhs[:, 512:670], start=st, stop=sp)
                        s_sb = gw.tile([128, HEADS], F32, tag="s")
                        s_cols = _bc(psum_g[:, F_HEAD:F_HEAD + 1],
                                     [[F67, HEADS]])
                        nc.vector.tensor_scalar(s_sb[:], s_cols, 1e-6,
                                                None, OP.max)
                        rs = gw.tile([128, HEADS], F32, tag="rs")
                        nc.vector.reciprocal(rs[:], s_sb[:])
                        A_norm = gw.tile([128, 704], BF, tag="anorm")
                        nc.gpsimd.memset(A_norm[:, 660:704], 0.0)
                        for h in range(HEADS):
                            nc.scalar.activation(
                                A_norm[:, 66 * h:66 * (h + 1)],
                                psum_g[:, F67 * h:F67 * h + F_HEAD],
                                AF.Copy, scale=rs[:, h:h + 1])
                        psum_h1 = psh.tile([128, 330], F32, tag="psh1")
                        psum_h2 = psh.tile([128, 330], F32, tag="psh2")
                        for h in range(HEADS):
                            ph = psum_h1 if h < 5 else psum_h2
                            o = 66 * h - (0 if h < 5 else 330)
                            tp = pst.tile([96, 128], BF, tag="tp")
                            nc.tensor.transpose(
                                tp[:], A_norm[:, 66 * h:66 * h + 96], ident_sb[:])
                            ahT = gw.tile([96, 128], BF, tag="ahT")
                            if h % 2 == 0:
                                nc.vector.tensor_copy(ahT[0:66, :], tp[0:66, :])
                            else:
                                nc.scalar.activation(ahT[0:66, :], tp[0:66, :],
                                                     AF.Copy)
                            nc.tensor.matmul(ph[:, o:o + 66], ahT[0:66, :],
                                             W_heads_sb[:, 66 * h:66 * (h + 1)],
                                             start=True, stop=False)
                            nc.tensor.matmul(ph[:, o:o + 66], ones_sb[0:1, 0:128],
                                             b_gat_sb[:, 66 * h:66 * (h + 1)],
                                             start=False, stop=True)
                        h_tile = gw.tile([128, F_GAT], BF, tag="htile")
                        for half, ph in ((0, psum_h1), (1, psum_h2)):
                            leaky_act(h_tile[:, 330 * half:330 * (half + 1)],
                                      ph[:, 0:330], al001, lkb)
                        nc.sync.dma_start(
                            h_my[128 * t:128 * (t + 1), 0:F_GAT], h_tile[:])

                        # piecewise AllGather: ship each finished piece while
                        # the remaining tiles compute
                        pend = np.cumsum(PIECES)
                        if t + 1 in pend:
                            p = int(np.searchsorted(pend, t + 1))
                            r0 = (int(pend[p]) - PIECES[p]) * 128
                            r1 = int(pend[p]) * 128
                            if collective:
                                nc.gpsimd.collective_compute(
                                    "AllGather", OP.bypass,
                                    replica_groups=[core_ids],
                                    ins=[h_my[r0:r1, :]],
                                    outs=[h_fp[p]])
                            else:
                                # sim-only stand-in keeping the dependencies
                                nc.gpsimd.dma_start(
                                    h_fp[p][0:r1 - r0, :], h_my[r0:r1, :])

                # ---------------- phase C: GCN + feature-major MLP -------------
                fpieces = [(128 * p, min(128, F_GAT - 128 * p))
                           for p in range(6)]
                with tc.tile_pool(name="phaseC", bufs=1) as pcw, \
                     tc.tile_pool(name="hg_pool", bufs=2) as hgp, \
                     tc.tile_pool(name="hgb_pool", bufs=1) as hgb, \
                     tc.tile_pool(name="gcn_work",
                                  bufs=4 if use_prelu else 2) as gcw, \
                     tc.tile_pool(name="aggTp", bufs=2) as aggp, \
                     tc.tile_pool(name="grp", bufs=1) as grp, \
                     tc.tile_pool(name="psumC", bufs=1,
                                  space=bass.MemorySpace.PSUM) as psc, \
                     tc.tile_pool(name="psumM", bufs=2,
                                  space=bass.MemorySpace.PSUM) as psm:

                    W_g1_sb = pcw.tile([128, 11 * 1000], BF)
                    nc.scalar.dma_start(W_g1_sb[:], W_g1_d[:])

                    groups = [(0, 4), (4, 4), (8, 2)]
                    kws = [128] * 5 + [20]
                    kws1 = [128] * 10 + [40]
                    kws2 = [128] * 7 + [104]
                    for g0, gn in groups:
                        nw = gn * 128
                        aggT = aggp.tile([128, 6 * 512], BF, tag="aggT")
                        for j in range(gn):
                            t = g0 + j
                            # one dma_gather per half-tile: the idx vector
                            # is chunk-major so the output lands [128, j, HW]
                            o = t * sum(gcols)
                            hgA = hgp.tile([128, h1 * HW], BF, tag="hgA")
                            nc.gpsimd.dma_gather(
                                hgA[:].rearrange("p (j e) -> p j e", e=HW),
                                h_full[:], sidx_sb[:, o:o + gcols[0]],
                                h1 * 128, h1 * 128, HW)
                            hgB = hgb.tile([128, (nc_t - h1) * HW], BF,
                                           tag="hgB")
                            nc.gpsimd.dma_gather(
                                hgB[:].rearrange("p (j e) -> p j e", e=HW),
                                h_full[:],
                                sidx_sb[:, o + gcols[0]:o + sum(gcols)],
                                (nc_t - h1) * 128, (nc_t - h1) * 128, HW)
                            psT = [psc.tile([128, 128], F32,
                                            tag=f"psT{p}", name=f"psT{p}")
                                   for p in range(6)]
                            for k in range(nc_t):
                                c = t * nc_t + k
                                wmask = gcw.tile([128, 128], BF, tag="wmask")
                                nc.vector.tensor_scalar(
                                    wmask[:], masks_sb[:, 128 * c:128 * (c + 1)],
                                    wvals_sb[:, c:c + 1], None, OP.mult)
                                hg = hgA if k < h1 else hgB
                                hcol = (k if k < h1 else k - h1) * HW
                                st, sp = (k == 0), (k == nc_t - 1)
                                for p, (f0, fw) in enumerate(fpieces):
                                    nc.tensor.matmul(
                                        psT[p][0:fw, :],
                                        hg[:, hcol + f0:hcol + f0 + fw],
                                        wmask[:], start=st, stop=sp)
                            for p, (f0, fw) in enumerate(fpieces):
                                nc.scalar.activation(
                                    aggT[0:fw, 512 * p + 128 * j:
                                         512 * p + 128 * (j + 1)],
                                    psT[p][0:fw, :], AF.Copy)

                        # ---- feature-major dense stack on this node group ----
                        gT = grp.tile([128, 11 * 512], BF, tag="gT")
                        for mc in range(11):
                            mw = 128 if mc < 10 else 40
                            ps = psm.tile([128, 512], F32, tag="psm")
                            for kt in range(6):
                                nc.tensor.matmul(
                                    ps[0:mw, 0:nw],
                                    W_gcn_sb[0:kws[kt],
                                             GCN_OUT * kt + 128 * mc:
                                             GCN_OUT * kt + 128 * mc + mw],
                                    aggT[0:kws[kt], 512 * kt:512 * kt + nw],
                                    start=(kt == 0), stop=(kt == 5))
                            leaky_act(gT[0:mw, 512 * mc:512 * mc + nw],
                                      ps[0:mw, 0:nw], al001, gcw,
                                      bias=b_gcn_sb[0:mw, mc:mc + 1], nparts=mw)

                        z1T = grp.tile([128, 8 * 512], BF, tag="z1T")
                        for mc in range(8):
                            mw = 128 if mc < 7 else 104
                            ps = psm.tile([128, 512], F32, tag="psm")
                            for kt in range(11):
                                nc.tensor.matmul(
                                    ps[0:mw, 0:nw],
                                    W_g1_sb[0:kws1[kt],
                                            1000 * kt + 128 * mc:
                                            1000 * kt + 128 * mc + mw],
                                    gT[0:kws1[kt], 512 * kt:512 * kt + nw],
                                    start=(kt == 0), stop=(kt == 10))
                            leaky_act(z1T[0:mw, 512 * mc:512 * mc + nw],
                                      ps[0:mw, 0:nw], al001, gcw,
                                      bias=b_g1_sb[0:mw, mc:mc + 1], nparts=mw)

                        ps2 = psm.tile([128, 512], F32, tag="psm")
                        for kt in range(8):
                            nc.tensor.matmul(
                                ps2[0:64, 0:nw],
                                W_g2_sb[0:kws2[kt], 64 * kt:64 * kt + 64],
                                z1T[0:kws2[kt], 512 * kt:512 * kt + nw],
                                start=(kt == 0), stop=(kt == 7))
                        z2T = gcw.tile([64, 512], BF, tag="z2T")
                        leaky_act(z2T[0:64, 0:nw], ps2[0:64, 0:nw], al001, gcw,
                                  bias=b_tail_sb[0:64, 0:1], nparts=64)

                        ps3 = psm.tile([128, 512], F32, tag="psm")
                        nc.tensor.matmul(ps3[0:32, 0:nw], W_fc1_sb[:],
                                         z2T[0:64, 0:nw], start=True, stop=True)
                        z3T = gcw.tile([32, 512], BF, tag="z3T")
                        leaky_act(z3T[0:32, 0:nw], ps3[0:32, 0:nw], al001, gcw,
                                  bias=b_tail_sb[0:32, 1:2], nparts=32)

                        ps4 = psm.tile([128, 512], F32, tag="psm")
                        nc.tensor.matmul(ps4[0:16, 0:nw], W_fc2_sb[:],
                                         z3T[0:32, 0:nw], start=True, stop=True)
                        z4T = gcw.tile([16, 512], BF, tag="z4T")
                        leaky_act(z4T[0:16, 0:nw], ps4[0:16, 0:nw], al001, gcw,
                                  bias=b_tail_sb[0:16, 2:3], nparts=16)

                        ps5 = psm.tile([128, 512], F32, tag="psm")
                        nc.tensor.matmul(ps5[0:1, 0:nw], W_out_sb[:],
                                         z4T[0:16, 0:nw], start=True, stop=True)
                        outT = gcw.tile([1, 512], F32, tag="outT")
                        nc.scalar.activation(outT[0:1, 0:nw], ps5[0:1, 0:nw],
                                             AF.Identity,
                                             bias=b_tail_sb[0:1, 3:4])
                        nc.sync.dma_start(y_d[0:1, 128 * g0:128 * g0 + nw],
                                          outT[0:1, 0:nw])

    nc.compile()
    return nc


# ---------------------------------------------------------------- entry point

SHARED_KEYS = ["w_asd", "w_as_bf", "W_heads", "b_gat_row", "ones_row", "ident",
               "W_gcn_p", "W_g1_p", "W_g2_p", "b_gcn_col", "b_g1_col",
               "W_fc1_p", "W_fc2_p", "W_out_p", "b_tail"]
PER_CORE_KEYS = ["xg", "xgT", "masks", "masksT", "wvals", "sidx", "xT_my"]


def kernel(x, edge_index, W_gat, att_src, att_dst, b_gat, W_gcn, b_gcn,
           W_g1, b_g1, W_g2, b_g2, W_fc1, b_fc1, W_fc2, b_fc2, W_out, b_out,
           _want_trace=False):
    x = np.asarray(x, np.float32)
    edge_index = np.asarray(edge_index)
    prep = _prep(x, edge_index)
    wts = _prep_weights(W_gat, att_src, att_dst, b_gat, W_gcn, b_gcn,
                        W_g1, b_g1, W_g2, b_g2, W_fc1, b_fc1, W_fc2, b_fc2,
                        W_out, b_out)

    nc_t = prep["nc_t"]
    use_prelu = not os.environ.get("NO_PRELU")
    key = (nc_t, use_prelu)
    if key not in _CACHE:
        _CACHE[key] = _build(nc_t, use_prelu=use_prelu)
    nc = _CACHE[key]

    shared = {k: wts[k] for k in SHARED_KEYS}
    in_maps = []
    for c in range(NCORE):
        m = dict(shared)
        for k in PER_CORE_KEYS:
            m[k] = prep[k][c]
        in_maps.append(m)

    res = run_bass_kernel_spmd(nc, in_maps, list(range(NCORE)),
                               trace=_want_trace)
    y_all = np.concatenate([np.asarray(res.results[c]["y"]).reshape(-1)
                            for c in range(NCORE)])
    out = y_all[prep["slot"]].astype(np.float32).reshape(N, 1)
    if _want_trace:
        return out, res
    return out


if __name__ == "__main__":
    sys.path.insert(0, os.path.dirname(os.path.abspath(__file__)))
    import reference
    inputs = reference.setup_inputs()
    inputs = {k: np.asarray(v) for k, v in inputs.items()}
    expected = np.asarray(reference.reference(**inputs))
    got = kernel(**inputs)
    err = np.linalg.norm(got - expected) / np.linalg.norm(expected)
    print("Relative error:", err)


# revision 51
# speedup vs baseline: 2.2403x; 2.2403x over previous
"""Trainium2 Bass kernel for GAT+GCN+MLP message passing (8 NeuronCores, SPMD).

Strategy (dst-node sharding), v2:
  - Host: add self-loops, greedily pack the 10000 nodes into 8 cores x 10
    tiles x 128 slots balancing in-edge counts; build per-edge one-hot dst
    masks (plus transposed variant) and the edge-expanded x operands.
  - GAT: per tile, all e-coefficients accumulate into one PSUM strip and go
    through a single Prelu+Exp on the scalar engine; the 670-wide per-edge
    message product (a constant-1 67th feature makes the softmax denominators
    fall out of the same aggregation matmul) is split across the vector and
    pool engines; the normalized aggregate is transposed per head for the
    per-head W matmul and leaky+bias is fused into scalar-engine Prelu ops.
  - The bf16 hidden h (660 wide) is exchanged with piecewise AllGathers
    (3/3/2/2 tiles each, issued as phase B finishes each piece) into row
    slices of one Local DRAM tensor so most of the collective hides under
    GAT compute; GCN gathers h rows one edge-chunk per indirect DMA
    (hardware honors only one offset column), aggregates feature-major
    (gathered rows stationary, on-device norm-scaled masks moving) so no
    transposes are needed, and the dense MLP runs feature-major with
    bias+leaky fused into Prelu activations.
"""

import os
import sys
import heapq
import dataclasses

for _p in ("/opt/trn_rl_repo", "/root/.axon_site/_ro/trn_rl_repo"):
    if os.path.isdir(_p) and _p not in sys.path:
        sys.path.insert(0, _p)

import numpy as np
import ml_dtypes

import concourse.bass as bass
import concourse.tile as tile
from concourse import bacc, mybir
from concourse.bass import IndirectOffsetOnAxis
from concourse.bass_utils import run_bass_kernel_spmd

BF16 = ml_dtypes.bfloat16

N = 10000
F_IN = 66
HEADS = 10
F_HEAD = 66
F_GAT = HEADS * F_HEAD          # 660
GCN_OUT = 1320
NCORE = 8
TILES_PER_CORE = 10
NTILE = NCORE * TILES_PER_CORE  # 80
NSLOT = NTILE * 128             # 10240
SLOTS_PER_CORE = TILES_PER_CORE * 128  # 1280
F67 = F_IN + 1                  # x features + a ones column (denominator trick)
HW = 660
PIECES = (3, 3, 2, 2)           # tiles per piecewise-AllGather piece

F32 = mybir.dt.float32
BF = mybir.dt.bfloat16
I32 = mybir.dt.int32

_CACHE = {}


# ---------------------------------------------------------------- host prep

def _prep(x, edge_index):
    src = np.concatenate([edge_index[0], np.arange(N, dtype=np.int64)])
    dst = np.concatenate([edge_index[1], np.arange(N, dtype=np.int64)])
    deg = np.bincount(dst, minlength=N).astype(np.int64)

    # pack nodes into 80 tiles of <=128 slots, balancing in-edge counts
    order = np.argsort(-deg, kind="stable")
    tile_cnt = np.zeros(NTILE, np.int64)
    slot = np.empty(N, np.int64)
    hp = [(0, t) for t in range(NTILE)]
    heapq.heapify(hp)
    for n_ in order:
        while True:
            e, t = heapq.heappop(hp)
            if tile_cnt[t] < 128:
                break
        slot[n_] = t * 128 + tile_cnt[t]
        tile_cnt[t] += 1
        heapq.heappush(hp, (e + int(deg[n_]), t))
    sslot = slot[src]
    dslot = slot[dst]
    dtile = dslot >> 7
    dlocal = dslot & 127

    tile_edges = np.bincount(dtile, minlength=NTILE)
    nc_t = int(np.max((tile_edges + 127) // 128))
    e_tile = nc_t * 128
    nchunks = TILES_PER_CORE * nc_t

    esrc = np.zeros((NCORE, TILES_PER_CORE, e_tile), np.int64)
    edstl = np.full((NCORE, TILES_PER_CORE, e_tile), -1, np.int64)
    edst = np.zeros((NCORE, TILES_PER_CORE, e_tile), np.int64)
    ord_t = np.argsort(dtile, kind="stable")
    bounds = np.searchsorted(dtile[ord_t], np.arange(NTILE + 1))
    for t in range(NTILE):
        idx = ord_t[bounds[t]:bounds[t + 1]]
        k = len(idx)
        c, tt = divmod(t, TILES_PER_CORE)
        esrc[c, tt, :k] = sslot[idx]
        edstl[c, tt, :k] = dlocal[idx]
        edst[c, tt, :k] = dslot[idx]

    # one-hot masks [core][128 edge-part, nchunks*128]
    onehot = (edstl[..., None] == np.arange(128))      # [C,T,e_tile,128] bool
    oh = onehot.reshape(NCORE, TILES_PER_CORE, nc_t, 128, 128)
    masks = np.ascontiguousarray(
        oh.transpose(0, 3, 1, 2, 4)).reshape(
        NCORE, 128, nchunks * 128).astype(BF16)
    masksT = np.ascontiguousarray(
        oh.transpose(0, 4, 1, 2, 3)).reshape(
        NCORE, 128, nchunks * 128).astype(BF16)

    # per-edge GCN norm weights, chunk-major [128, nchunks]
    dinv_slot = np.ones(NSLOT, np.float32)
    dinv_slot[slot] = 1.0 / np.sqrt(np.maximum(deg, 1).astype(np.float32))
    wv = (dinv_slot[esrc] * dinv_slot[edst]).astype(np.float32)
    wvals = np.ascontiguousarray(
        wv.reshape(NCORE, nchunks, 128).transpose(0, 2, 1))

    # edge-expanded x operands; xg carries a constant-1 67th feature so the
    # softmax denominators fall out of the same aggregation matmul
    x_pad = np.zeros((NSLOT, F_IN), np.float32)
    x_pad[slot] = x
    xg = np.empty((NCORE, 128, nchunks * F67), BF16)
    xgT = np.empty((NCORE, F_IN, nchunks * 128), BF16)
    for c in range(NCORE):
        arr = x_pad[esrc[c].reshape(-1)]               # [nidx, 66] f32
        a3 = arr.reshape(nchunks, 128, F_IN)
        a4 = np.concatenate(
            [a3, np.ones((nchunks, 128, 1), np.float32)], axis=2)
        xg[c] = np.ascontiguousarray(
            a4.transpose(1, 0, 2)).reshape(128, nchunks * F67).astype(BF16)
        xgT[c] = np.ascontiguousarray(arr.T).astype(BF16)

    # remap slot ids to the piece-major h_full layout produced by the
    # piecewise AllGather: pieces of PIECES tiles; piece p holds
    # [core0 rows | core1 rows | ...], rows = (tile - first_tile_p)*128+local
    pbase = np.zeros(len(PIECES) + 1, np.int64)
    for p, ntl in enumerate(PIECES):
        pbase[p + 1] = pbase[p] + NCORE * ntl * 128
    tile2piece = np.repeat(np.arange(len(PIECES)), PIECES)
    tile2first = np.concatenate(
        [[sum(PIECES[:p])] * PIECES[p] for p in range(len(PIECES))])

    piece_arr = np.asarray(PIECES, np.int64)

    def remap(slots):
        core = slots // SLOTS_PER_CORE
        tl = (slots % SLOTS_PER_CORE) // 128
        local = slots % 128
        p = tile2piece[tl]
        return (pbase[p] + core * (piece_arr[p] * 128)
                + (tl - tile2first[tl]) * 128 + local)

    sidx = np.empty((NCORE, 128, nchunks), np.int32)
    for c in range(NCORE):
        a = remap(esrc[c]).reshape(
            TILES_PER_CORE, nc_t, 128).transpose(2, 0, 1)
        sidx[c] = a.reshape(128, nchunks)

    xT = np.zeros((F_IN, NSLOT), np.float32)
    xT[:, slot] = x.T
    xT_my = np.stack([np.ascontiguousarray(
        xT[:, c * SLOTS_PER_CORE:(c + 1) * SLOTS_PER_CORE])
        for c in range(NCORE)])

    return dict(slot=slot, nc_t=nc_t, masks=masks, masksT=masksT,
                wvals=wvals, xg=xg, xgT=xgT, sidx=sidx, xT_my=xT_my)


def _prep_weights(W_gat, att_src, att_dst, b_gat, W_gcn, b_gcn,
                  W_g1, b_g1, W_g2, b_g2, W_fc1, b_fc1, W_fc2, b_fc2,
                  W_out, b_out):
    Wg = np.asarray(W_gat, np.float32).reshape(F_IN, HEADS, F_HEAD)
    w_as = np.einsum("fhg,hg->fh", Wg, np.asarray(att_src, np.float32))
    w_ad = np.einsum("fhg,hg->fh", Wg, np.asarray(att_dst, np.float32))
    w_asd = np.ascontiguousarray(
        np.concatenate([w_as, w_ad], axis=1).astype(np.float32))  # [66,20]
    w_as_bf = np.ascontiguousarray(w_as.astype(BF16))             # [66,10]

    def chunk_pack(W, kchunks, ncols):
        W = np.asarray(W, np.float32)
        K, M = W.shape
        out = np.zeros((128, kchunks * ncols), BF16)
        for kt in range(kchunks):
            r0 = kt * 128
            r1 = min(K, r0 + 128)
            if r0 >= K:
                break
            out[:r1 - r0, kt * ncols:kt * ncols + M] = W[r0:r1].astype(BF16)
        return out

    W_gcn_p = chunk_pack(W_gcn, 6, GCN_OUT)
    W_g1_p = chunk_pack(W_g1, 11, 1000)
    W_g2_p = chunk_pack(W_g2, 8, 64)

    def col_pack(b, nch):
        b = np.asarray(b, np.float32).reshape(-1)
        out = np.zeros((128, nch), np.float32)
        for mc in range(nch):
            r0 = mc * 128
            r1 = min(b.shape[0], r0 + 128)
            if r0 >= b.shape[0]:
                break
            out[:r1 - r0, mc] = b[r0:r1]
        return out

    b_gcn_col = col_pack(b_gcn, 11)
    b_g1_col = col_pack(b_g1, 8)
    W_fc1_p = np.asarray(W_fc1, BF16)
    W_fc2_p = np.asarray(W_fc2, BF16)
    W_out_p = np.asarray(W_out, BF16)
    b_tail = np.zeros((128, 4), np.float32)
    b_tail[:64, 0] = np.asarray(b_g2, np.float32)
    b_tail[:32, 1] = np.asarray(b_fc1, np.float32)
    b_tail[:16, 2] = np.asarray(b_fc2, np.float32)
    b_tail[0, 3] = float(np.asarray(b_out).reshape(-1)[0])

    ident = np.eye(128, dtype=BF16)
    ones_row = np.ones((1, 512), BF16)
    b_gat_row = np.zeros((1, F_GAT), BF16)
    b_gat_row[0, :] = np.asarray(b_gat, BF16)
    W_heads = np.asarray(W_gat, BF16)

    return dict(w_asd=w_asd, w_as_bf=w_as_bf, W_gcn_p=W_gcn_p, W_g1_p=W_g1_p,
                W_g2_p=W_g2_p, b_gcn_col=b_gcn_col, b_g1_col=b_g1_col,
                W_fc1_p=W_fc1_p, W_fc2_p=W_fc2_p, W_out_p=W_out_p,
                b_tail=b_tail, ident=ident, ones_row=ones_row,
                b_gat_row=b_gat_row, W_heads=W_heads)


# ---------------------------------------------------------------- device kernel

def _bc(ap, pattern):
    """Replace the free dims of a (sliced) AP with explicit [step,count] dims."""
    return dataclasses.replace(
        ap, ap=[list(ap.ap[0])] + [list(p) for p in pattern])


def _build(nc_t, repeat=1, collective=True, use_prelu=True):
    nchunks = TILES_PER_CORE * nc_t

    nc = bacc.Bacc("TRN2", target_bir_lowering=False, debug=False,
                   num_devices=NCORE, dynamic_dma_scratch_size=1 << 16)

    def inp(name, shape, dt):
        return nc.dram_tensor(name, list(shape), dt, kind="ExternalInput")

    xg_d = inp("xg", [128, nchunks * F67], BF)
    xgT_d = inp("xgT", [F_IN, nchunks * 128], BF)
    masks_d = inp("masks", [128, nchunks * 128], BF)
    masksT_d = inp("masksT", [128, nchunks * 128], BF)
    wvals_d = inp("wvals", [128, nchunks], F32)
    sidx_d = inp("sidx", [128, nchunks], I32)
    xTmy_d = inp("xT_my", [F_IN, SLOTS_PER_CORE], F32)
    w_asd_d = inp("w_asd", [F_IN, 2 * HEADS], F32)
    w_as_bf_d = inp("w_as_bf", [F_IN, HEADS], BF)
    W_heads_d = inp("W_heads", [F_IN, F_GAT], BF)
    b_gat_row_d = inp("b_gat_row", [1, F_GAT], BF)
    ones_row_d = inp("ones_row", [1, 512], BF)
    ident_d = inp("ident", [128, 128], BF)
    W_gcn_d = inp("W_gcn_p", [128, 6 * GCN_OUT], BF)
    W_g1_d = inp("W_g1_p", [128, 11 * 1000], BF)
    W_g2_d = inp("W_g2_p", [128, 8 * 64], BF)
    b_gcn_col_d = inp("b_gcn_col", [128, 11], F32)
    b_g1_col_d = inp("b_g1_col", [128, 8], F32)
    W_fc1_d = inp("W_fc1_p", [64, 32], BF)
    W_fc2_d = inp("W_fc2_p", [32, 16], BF)
    W_out_d = inp("W_out_p", [16, 1], BF)
    b_tail_d = inp("b_tail", [128, 4], F32)

    y_d = nc.dram_tensor("y", [1, SLOTS_PER_CORE], F32, kind="ExternalOutput")

    core_ids = list(range(NCORE))
    AF = mybir.ActivationFunctionType
    OP = mybir.AluOpType

    with tile.TileContext(nc) as tc:
        with tc.tile_pool(name="persist", bufs=1) as pp, \
             tc.tile_pool(name="dram", bufs=1, space="DRAM") as dram:

            al001 = pp.tile([128, 1], F32)
            nc.vector.memset(al001[:], 0.01)
            al02 = pp.tile([128, 1], F32)
            nc.vector.memset(al02[:], 0.2)

            def leaky_act(out_ap, in_ap, alpha_ap, pool, bias=None, nparts=128):
                """out = leaky(in + bias); bias is a per-partition AP or None."""
                if use_prelu:
                    kw = {}
                    if bias is not None:
                        kw["bias"] = bias
                    nc.scalar.activation(out_ap, in_ap, AF.Prelu,
                                         alpha=alpha_ap[0:nparts, 0:1], **kw)
                else:
                    fs = in_ap.free_size()
                    th = pool.tile([128, fs], BF, tag="lk_t", name="lk_t")
                    if bias is not None:
                        nc.scalar.activation(th[0:nparts, 0:fs], in_ap,
                                             AF.Identity, bias=bias)
                    else:
                        nc.scalar.activation(th[0:nparts, 0:fs], in_ap, AF.Copy)
                    u = pool.tile([128, fs], BF, tag="lk_u", name="lk_u")
                    alpha = 0.2 if alpha_ap is al02 else 0.01
                    nc.vector.tensor_scalar(u[0:nparts, 0:fs], th[0:nparts, 0:fs],
                                            alpha, None, OP.mult)
                    nc.vector.tensor_tensor(out_ap, th[0:nparts, 0:fs],
                                            u[0:nparts, 0:fs], OP.max)

            sidx_sb = pp.tile([128, nchunks], I32)
            ident_sb = pp.tile([128, 128], BF)
            ones_sb = pp.tile([1, 512], BF)
            wvals_sb = pp.tile([128, nchunks], F32)
            asd_my = pp.tile([128, TILES_PER_CORE * 2 * HEADS], F32)
            asd_my_bf = pp.tile([128, TILES_PER_CORE * 2 * HEADS], BF)
            nc.sync.dma_start(sidx_sb[:], sidx_d[:])
            nc.sync.dma_start(ident_sb[:], ident_d[:])
            nc.sync.dma_start(ones_sb[:], ones_row_d[:])
            nc.sync.dma_start(wvals_sb[:], wvals_d[:])

            # phase-C weights: load early on the vector queue so they stream
            # behind the phase-B inputs
            W_gcn_sb = pp.tile([128, 6 * GCN_OUT], BF)
            nc.scalar.dma_start(W_gcn_sb[:], W_gcn_d[:])
            W_g2_sb = pp.tile([128, 8 * 64], BF)
            nc.scalar.dma_start(W_g2_sb[:], W_g2_d[:])
            b_gcn_sb = pp.tile([128, 11], F32)
            nc.scalar.dma_start(b_gcn_sb[:], b_gcn_col_d[:])
            b_g1_sb = pp.tile([128, 8], F32)
            nc.scalar.dma_start(b_g1_sb[:], b_g1_col_d[:])
            W_fc1_sb = pp.tile([64, 32], BF)
            nc.scalar.dma_start(W_fc1_sb[:], W_fc1_d[:])
            W_fc2_sb = pp.tile([32, 16], BF)
            nc.scalar.dma_start(W_fc2_sb[:], W_fc2_d[:])
            W_out_sb = pp.tile([16, 1], BF)
            nc.scalar.dma_start(W_out_sb[:], W_out_d[:])
            b_tail_sb = pp.tile([128, 4], F32)
            nc.scalar.dma_start(b_tail_sb[:], b_tail_d[:])

            # masks persist through both phases (phase C derives wmasks);
            # loaded tile-interleaved alongside the phase-B inputs below
            masks_sb = pp.tile([128, nchunks * 128], BF)

            for _rep in range(repeat):
                h_my = dram.tile([SLOTS_PER_CORE, HW], BF,
                                 name=f"h_my_{_rep}")
                h_full = dram.tile([NSLOT, HW], BF,
                                   name=f"h_full_{_rep}")
                pb = 0
                h_fp = []
                for p, ntl in enumerate(PIECES):
                    h_fp.append(h_full[pb:pb + NCORE * ntl * 128, :])
                    pb += NCORE * ntl * 128
                # ---------------- phase A: own-node attention coefficients ----
                with tc.tile_pool(name="phaseA", bufs=1) as pa, \
                     tc.tile_pool(name="psumA", bufs=4,
                                  space=bass.MemorySpace.PSUM) as psa:
                    xTmy_sb = pa.tile([F_IN, SLOTS_PER_CORE], F32)
                    nc.sync.dma_start(xTmy_sb[:], xTmy_d[:])
                    wasd_sb = pa.tile([F_IN, 2 * HEADS], F32)
                    nc.sync.dma_start(wasd_sb[:], w_asd_d[:])
                    for t in range(TILES_PER_CORE):
                        ps = psa.tile([128, 2 * HEADS], F32, tag="psA")
                        nc.tensor.matmul(ps[:], xTmy_sb[:, 128 * t:128 * (t + 1)],
                                         wasd_sb[:], start=True, stop=True)
                        nc.vector.tensor_copy(
                            asd_my[:, 20 * t:20 * (t + 1)], ps[:])
                        nc.vector.tensor_copy(
                            asd_my_bf[:, 20 * t:20 * (t + 1)], ps[:])

                # ---------------- phase B: GAT ----------------
                with tc.tile_pool(name="phaseB", bufs=1) as pb, \
                     tc.tile_pool(name="gat_work",
                                  bufs=4 if use_prelu else 3) as gw, \
                     tc.tile_pool(name="lk_b", bufs=2) as lkb, \
                     tc.tile_pool(name="psumE", bufs=1,
                                  space=bass.MemorySpace.PSUM) as pse, \
                     tc.tile_pool(name="psumG", bufs=2,
                                  space=bass.MemorySpace.PSUM) as psg, \
                     tc.tile_pool(name="psumH", bufs=1,
                                  space=bass.MemorySpace.PSUM) as psh, \
                     tc.tile_pool(name="psumT", bufs=1,
                                  space=bass.MemorySpace.PSUM) as pst:

                    xg_sb = pb.tile([128, nchunks * F67], BF)
                    xgT_sb = pb.tile([F_IN, nchunks * 128], BF)
                    masksT_sb = pb.tile([128, nchunks * 128], BF)
                    was_sb = pb.tile([F_IN, HEADS], BF)
                    nc.sync.dma_start(was_sb[:], w_as_bf_d[:])
                    W_heads_sb = pb.tile([F_IN, F_GAT], BF)
                    nc.sync.dma_start(W_heads_sb[:], W_heads_d[:])
                    b_gat_sb = pb.tile([1, F_GAT], BF)
                    nc.sync.dma_start(b_gat_sb[:], b_gat_row_d[:])
                    for t in range(TILES_PER_CORE):
                        sl = slice(128 * nc_t * t, 128 * nc_t * (t + 1))
                        nc.sync.dma_start(xgT_sb[:, sl], xgT_d[:, sl])
                        nc.sync.dma_start(masksT_sb[:, sl], masksT_d[:, sl])
                        slx = slice(F67 * nc_t * t, F67 * nc_t * (t + 1))
                        nc.sync.dma_start(xg_sb[:, slx], xg_d[:, slx])
                        nc.sync.dma_start(
                            masks_sb[:, sl],
                            masks_d[:, sl])
                    exb = pb.tile([128, nchunks * HEADS], BF)

                    for t in range(TILES_PER_CORE):
                        a_d_tile = asd_my_bf[:, 20 * t + 10:20 * (t + 1)]
                        # e-values for the whole tile in one PSUM strip
                        ps_e = pse.tile([128, nc_t * HEADS], F32, tag="pse")
                        for k in range(nc_t):
                            c = t * nc_t + k
                            nc.tensor.matmul(ps_e[:, 10 * k:10 * (k + 1)],
                                             xgT_sb[:, 128 * c:128 * (c + 1)],
                                             was_sb[:], start=True, stop=False)
                            nc.tensor.matmul(ps_e[:, 10 * k:10 * (k + 1)],
                                             masksT_sb[:, 128 * c:128 * (c + 1)],
                                             a_d_tile, start=False, stop=True)
                        ev = gw.tile([128, nc_t * HEADS], F32, tag="ev")
                        if use_prelu:
                            nc.scalar.activation(ev[:], ps_e[:], AF.Prelu,
                                                 alpha=al02[:, 0:1])
                        else:
                            nc.vector.tensor_scalar(ev[:], ps_e[:], 0.2, None,
                                                    OP.mult)
                            nc.vector.tensor_tensor(ev[:], ps_e[:], ev[:], OP.max)
                        nc.scalar.activation(
                            exb[:, 10 * nc_t * t:10 * nc_t * (t + 1)], ev[:],
                            AF.Exp)

                        # psum_g: aggregate at cols [0:660), denominators [660:670)
                        psum_g = psg.tile([128, 670], F32, tag="psg")
                        for k in range(nc_t):
                            c = t * nc_t + k
                            rhs = gw.tile([128, 10 * F67], BF, tag="rhs")
                            xg_b1 = _bc(xg_sb[:, F67 * c:F67 * c + 1],
                                        [[0, 7], [1, F67]])
                            ex_b1 = _bc(exb[:, HEADS * c:HEADS * c + 1],
                                        [[1, 7], [0, F67]])
                            nc.vector.tensor_tensor(
                                rhs[:, 0:469].rearrange("p (h f) -> p h f", h=7),
                                xg_b1, ex_b1, OP.mult)
                            xg_b2 = _bc(xg_sb[:, F67 * c:F67 * c + 1],
                                        [[0, 3], [1, F67]])
                            ex_b2 = _bc(exb[:, HEADS * c + 7:HEADS * c + 8],
                                        [[1, 3], [0, F67]])
                            nc.gpsimd.tensor_tensor(
                                rhs[:, 469:670].rearrange("p (h f) -> p h f", h=3),
                                xg_b2, ex_b2, OP.mult)
                            mask = masks_sb[:, 128 * c:128 * (c + 1)]
                            st, sp = (k == 0), (k == nc_t - 1)
                            nc.tensor.matmul(psum_g[:, 0:512], mask,
                                             rhs[:, 0:512], start=st, stop=sp)
                            nc.tensor.matmul(psum_g[:, 512:670], mask,
                                             rhs[:, 512:670], start=st, stop=sp)
                        s_sb = gw.tile([128, HEADS], F32, tag="s")
                        s_cols = _bc(psum_g[:, F_HEAD:F_HEAD + 1],
                                     [[F67, HEADS]])
                        nc.vector.tensor_scalar(s_sb[:], s_cols, 1e-6,
                                                None, OP.max)
                        rs = gw.tile([128, HEADS], F32, tag="rs")
                        nc.vector.reciprocal(rs[:], s_sb[:])
                        A_norm = gw.tile([128, 704], BF, tag="anorm")
                        nc.gpsimd.memset(A_norm[:, 660:704], 0.0)
                        for h in range(HEADS):
                            nc.scalar.activation(
                                A_norm[:, 66 * h:66 * (h + 1)],
                                psum_g[:, F67 * h:F67 * h + F_HEAD],
                                AF.Copy, scale=rs[:, h:h + 1])
                        psum_h1 = psh.tile([128, 330], F32, tag="psh1")
                        psum_h2 = psh.tile([128, 330], F32, tag="psh2")
                        for h in range(HEADS):
                            ph = psum_h1 if h < 5 else psum_h2
                            o = 66 * h - (0 if h < 5 else 330)
                            tp = pst.tile([96, 128], BF, tag="tp")
                            nc.tensor.transpose(
                                tp[:], A_norm[:, 66 * h:66 * h + 96], ident_sb[:])
                            ahT = gw.tile([96, 128], BF, tag="ahT")
                            if h % 2 == 0:
                                nc.vector.tensor_copy(ahT[0:66, :], tp[0:66, :])
                            else:
                                nc.scalar.activation(ahT[0:66, :], tp[0:66, :],
                                                     AF.Copy)
                            nc.tensor.matmul(ph[:, o:o + 66], ahT[0:66, :],
                                             W_heads_sb[:, 66 * h:66 * (h + 1)],
                                             start=True, stop=False)
                            nc.tensor.matmul(ph[:, o:o + 66], ones_sb[0:1, 0:128],
                                             b_gat_sb[:, 66 * h:66 * (h + 1)],
                                             start=False, stop=True)
                        h_tile = gw.tile([128, HW], BF, tag="htile")
                        for half, ph in ((0, psum_h1), (1, psum_h2)):
                            leaky_act(h_tile[:, 330 * half:330 * (half + 1)],
                                      ph[:, 0:330], al001, lkb)
                        nc.sync.dma_start(
                            h_my[128 * t:128 * (t + 1), :], h_tile[:])

                        # piecewise AllGather: ship each finished piece while
                        # the remaining tiles compute
                        pend = np.cumsum(PIECES)
                        if t + 1 in pend:
                            p = int(np.searchsorted(pend, t + 1))
                            r0 = (int(pend[p]) - PIECES[p]) * 128
                            r1 = int(pend[p]) * 128
                            if collective:
                                nc.gpsimd.collective_compute(
                                    "AllGather", OP.bypass,
                                    replica_groups=[core_ids],
                                    ins=[h_my[r0:r1, :]],
                                    outs=[h_fp[p]])
                            else:
                                # sim-only stand-in keeping the dependencies
                                nc.gpsimd.dma_start(
                                    h_fp[p][0:r1 - r0, :], h_my[r0:r1, :])

                # ---------------- phase C: GCN + feature-major MLP -------------
                fpieces = [(128 * p, min(128, F_GAT - 128 * p))
                           for p in range(6)]
                with tc.tile_pool(name="phaseC", bufs=1) as pcw, \
                     tc.tile_pool(name="hg_pool", bufs=24) as hgp, \
                     tc.tile_pool(name="gcn_work",
                                  bufs=4 if use_prelu else 2) as gcw, \
                     tc.tile_pool(name="aggTp", bufs=2) as aggp, \
                     tc.tile_pool(name="grp", bufs=1) as grp, \
                     tc.tile_pool(name="psumC", bufs=1,
                                  space=bass.MemorySpace.PSUM) as psc, \
                     tc.tile_pool(name="psumM", bufs=2,
                                  space=bass.MemorySpace.PSUM) as psm:

                    W_g1_sb = pcw.tile([128, 11 * 1000], BF)
                    nc.scalar.dma_start(W_g1_sb[:], W_g1_d[:])

                    groups = [(0, 4), (4, 4), (8, 2)]
                    kws = [128] * 5 + [20]
                    kws1 = [128] * 10 + [40]
                    kws2 = [128] * 7 + [104]
                    for g0, gn in groups:
                        nw = gn * 128
                        aggT = aggp.tile([128, 6 * 512], BF, tag="aggT")
                        for j in range(gn):
                            t = g0 + j
                            # hw honors only one offset column per indirect
                            # DMA, so gather one chunk per descriptor batch,
                            # prefetched deep via the pool rotation
                            hgs = []
                            for k in range(nc_t):
                                c = t * nc_t + k
                                hg = hgp.tile([128, HW], BF, tag="hg")
                                nc.gpsimd.indirect_dma_start(
                                    hg[:], None, h_full[:],
                                    IndirectOffsetOnAxis(
                                        ap=sidx_sb[:, c:c + 1], axis=0))
                                hgs.append(hg)
                            psT = [psc.tile([128, 128], F32,
                                            tag=f"psT{p}", name=f"psT{p}")
                                   for p in range(6)]
                            for k in range(nc_t):
                                c = t * nc_t + k
                                wmask = gcw.tile([128, 128], BF, tag="wmask")
                                nc.vector.tensor_scalar(
                                    wmask[:], masks_sb[:, 128 * c:128 * (c + 1)],
                                    wvals_sb[:, c:c + 1], None, OP.mult)
                                hg = hgs[k]
                                st, sp = (k == 0), (k == nc_t - 1)
                                for p, (f0, fw) in enumerate(fpieces):
                                    nc.tensor.matmul(
                                        psT[p][0:fw, :],
                                        hg[:, f0:f0 + fw],
                                        wmask[:], start=st, stop=sp)
                            for p, (f0, fw) in enumerate(fpieces):
                                nc.scalar.activation(
                                    aggT[0:fw, 512 * p + 128 * j:
                                         512 * p + 128 * (j + 1)],
                                    psT[p][0:fw, :], AF.Copy)

                        # ---- feature-major dense stack on this node group ----
                        gT = grp.tile([128, 11 * 512], BF, tag="gT")
                        for mc in range(11):
                            mw = 128 if mc < 10 else 40
                            ps = psm.tile([128, 512], F32, tag="psm")
                            for kt in range(6):
                                nc.tensor.matmul(
                                    ps[0:mw, 0:nw],
                                    W_gcn_sb[0:kws[kt],
                                             GCN_OUT * kt + 128 * mc:
                                             GCN_OUT * kt + 128 * mc + mw],
                                    aggT[0:kws[kt], 512 * kt:512 * kt + nw],
                                    start=(kt == 0), stop=(kt == 5))
                            leaky_act(gT[0:mw, 512 * mc:512 * mc + nw],
                                      ps[0:mw, 0:nw], al001, gcw,
                                      bias=b_gcn_sb[0:mw, mc:mc + 1], nparts=mw)

                        z1T = grp.tile([128, 8 * 512], BF, tag="z1T")
                        for mc in range(8):
                            mw = 128 if mc < 7 else 104
                            ps = psm.tile([128, 512], F32, tag="psm")
                            for kt in range(11):
                                nc.tensor.matmul(
                                    ps[0:mw, 0:nw],
                                    W_g1_sb[0:kws1[kt],
                                            1000 * kt + 128 * mc:
                                            1000 * kt + 128 * mc + mw],
                                    gT[0:kws1[kt], 512 * kt:512 * kt + nw],
                                    start=(kt == 0), stop=(kt == 10))
                            leaky_act(z1T[0:mw, 512 * mc:512 * mc + nw],
                                      ps[0:mw, 0:nw], al001, gcw,
                                      bias=b_g1_sb[0:mw, mc:mc + 1], nparts=mw)

                        ps2 = psm.tile([128, 512], F32, tag="psm")
                        for kt in range(8):
                            nc.tensor.matmul(
                                ps2[0:64, 0:nw],
                                W_g2_sb[0:kws2[kt], 64 * kt:64 * kt + 64],
                                z1T[0:kws2[kt], 512 * kt:512 * kt + nw],
                                start=(kt == 0), stop=(kt == 7))
                        z2T = gcw.tile([64, 512], BF, tag="z2T")
                        leaky_act(z2T[0:64, 0:nw], ps2[0:64, 0:nw], al001, gcw,
                                  bias=b_tail_sb[0:64, 0:1], nparts=64)

                        ps3 = psm.tile([128, 512], F32, tag="psm")
                        nc.tensor.matmul(ps3[0:32, 0:nw], W_fc1_sb[:],
                                         z2T[0:64, 0:nw], start=True, stop=True)
                        z3T = gcw.tile([32, 512], BF, tag="z3T")
                        leaky_act(z3T[0:32, 0:nw], ps3[0:32, 0:nw], al001, gcw,
                                  bias=b_tail_sb[0:32, 1:2], nparts=32)

                        ps4 = psm.tile([128, 512], F32, tag="psm")
                        nc.tensor.matmul(ps4[0:16, 0:nw], W_fc2_sb[:],
                                         z3T[0:32, 0:nw], start=True, stop=True)
                        z4T = gcw.tile([16, 512], BF, tag="z4T")
                        leaky_act(z4T[0:16, 0:nw], ps4[0:16, 0:nw], al001, gcw,
                                  bias=b_tail_sb[0:16, 2:3], nparts=16)

                        ps5 = psm.tile([128, 512], F32, tag="psm")
                        nc.tensor.matmul(ps5[0:1, 0:nw], W_out_sb[:],
                                         z4T[0:16, 0:nw], start=True, stop=True)
                        outT = gcw.tile([1, 512], F32, tag="outT")
                        nc.scalar.activation(outT[0:1, 0:nw], ps5[0:1, 0:nw],
                                             AF.Identity,
                                             bias=b_tail_sb[0:1, 3:4])
                        nc.sync.dma_start(y_d[0:1, 128 * g0:128 * g0 + nw],
                                          outT[0:1, 0:nw])

    nc.compile()
    return nc


# ---------------------------------------------------------------- entry point

SHARED_KEYS = ["w_asd", "w_as_bf", "W_heads", "b_gat_row", "ones_row", "ident",
               "W_gcn_p", "W_g1_p", "W_g2_p", "b_gcn_col", "b_g1_col",
               "W_fc1_p", "W_fc2_p", "W_out_p", "b_tail"]
PER_CORE_KEYS = ["xg", "xgT", "masks", "masksT", "wvals", "sidx", "xT_my"]


def kernel(x, edge_index, W_gat, att_src, att_dst, b_gat, W_gcn, b_gcn,
           W_g1, b_g1, W_g2, b_g2, W_fc1, b_fc1, W_fc2, b_fc2, W_out, b_out,
           _want_trace=False):
    x = np.asarray(x, np.float32)
    edge_index = np.asarray(edge_index)
    prep = _prep(x, edge_index)
    wts = _prep_weights(W_gat, att_src, att_dst, b_gat, W_gcn, b_gcn,
                        W_g1, b_g1, W_g2, b_g2, W_fc1, b_fc1, W_fc2, b_fc2,
                        W_out, b_out)

    nc_t = prep["nc_t"]
    use_prelu = not os.environ.get("NO_PRELU")
    key = (nc_t, use_prelu)
    if key not in _CACHE:
        _CACHE[key] = _build(nc_t, use_prelu=use_prelu)
    nc = _CACHE[key]

    shared = {k: wts[k] for k in SHARED_KEYS}
    in_maps = []
    for c in range(NCORE):
        m = dict(shared)
        for k in PER_CORE_KEYS:
            m[k] = prep[k][c]
        in_maps.append(m)

    res = run_bass_kernel_spmd(nc, in_maps, list(range(NCORE)),
                               trace=_want_trace)
    y_all = np.concatenate([np.asarray(res.results[c]["y"]).reshape(-1)
                            for c in range(NCORE)])
    out = y_all[prep["slot"]].astype(np.float32).reshape(N, 1)
    if _want_trace:
        return out, res
    return out


if __name__ == "__main__":
    sys.path.insert(0, os.path.dirname(os.path.abspath(__file__)))
    import reference
    inputs = reference.setup_inputs()
    inputs = {k: np.asarray(v) for k, v in inputs.items()}
    expected = np.asarray(reference.reference(**inputs))
    got = kernel(**inputs)
    err = np.linalg.norm(got - expected) / np.linalg.norm(expected)
    print("Relative error:", err)
